# revision 8
# baseline (speedup 1.0000x reference)
"""Sliding-window attention (B=2, S=2048, D=1024, H=16, window=256) on 8
Trainium2 NeuronCores.

Sharding: 8 cores = 2 batches x 4 sequence quarters (512 queries each).
The 256-wide causal window makes attention local, so each core carries a
256-position left halo of x for K/V and produces a clean output shard --
no collectives and no cross-core reduction.

Per-core device kernel (transposed/feature-major layouts, zero on-chip
transposes), fully software-pipelined: V projection first, then per
feature-group g the q/k projections IMMEDIATELY followed by that group's
two attention heads, so scores/exp/mask/PV stream behind the remaining
projections on all five engines. Per head: scoresT[key, query] K=64
matmuls -> exp straight from PSUM on the scalar engine -> one
multiplicative trimask DVE op (triangles + sequence-edge validity,
host-built per core) -> attention x V with the softmax denominator fused
via a ones column in V (no zero-init: the first PV matmul's start=True
bank-clear + overwrite-on-clear handles initialization, HW-verified
semantics) -> reciprocal + gpsimd partition broadcast.
bf16 operands, fp32 PSUM accumulation; weights stream as contiguous
[128,1024] row chunks issued up front. HW-validated rel L2 err ~5e-3.
"""
from contextlib import ExitStack

import numpy as np
import ml_dtypes

import concourse.bass as bass
import concourse.mybir as mybir
import concourse.tile as tile
from concourse import bacc

F32 = mybir.dt.float32
BF16 = mybir.dt.bfloat16

D = 1024
H = 16
S_OWN = 512
HALO = 256
S_HALO = 768
NQT = 4
NKT = 6
P = 128

KT_SPANS = [(max(0, kt - 2), min(NQT - 1, kt)) for kt in range(NKT)]
KT_N = [128 * (hi - lo + 1) for (lo, hi) in KT_SPANS]
KT_OFF = np.concatenate([[0], np.cumsum(KT_N)]).tolist()

_STORE = {"f32": F32, "bf16": BF16}
_NP = {"f32": np.float32, "bf16": ml_dtypes.bfloat16}


def build_nc(proj_dt="bf16", att_dt="bf16", psS_bufs=4, psO_bufs=2, e_bufs=4, qk_bufs=2):
    pst = _STORE[proj_dt]
    ast = _STORE[att_dt]

    nc = bacc.Bacc("TRN2", target_bir_lowering=False, debug=False, num_devices=8)

    xT = nc.dram_tensor("xT", [D, S_HALO], pst, kind="ExternalInput").ap()
    wqT = nc.dram_tensor("wqT", [D, D], pst, kind="ExternalInput").ap()
    wkT = nc.dram_tensor("wkT", [D, D], pst, kind="ExternalInput").ap()
    wvT = nc.dram_tensor("wvT", [D, D], pst, kind="ExternalInput").ap()
    woT = nc.dram_tensor("woT", [D, D], pst, kind="ExternalInput").ap()
    trimd = nc.dram_tensor("trimask7", [P, 1536], ast, kind="ExternalInput").ap()
    yT = nc.dram_tensor("yT", [D, S_OWN], F32, kind="ExternalOutput").ap()

    with nc.allow_low_precision(reason="bf16 compute, f32 psum accumulation"), \
         tile.TileContext(nc) as tc, ExitStack() as ctx:
        const = ctx.enter_context(tc.tile_pool(name="const", bufs=1))
        persist = ctx.enter_context(tc.tile_pool(name="persist", bufs=1))

        def rowchunks(wT, name, width):
            ts_ = []
            for c in range(8):
                t = persist.tile([P, width], pst, tag=f"{name}{c}")
                nc.sync.dma_start(t[:], wT[c * P:(c + 1) * P, :])
                ts_.append(t)
            return ts_

        # interleave xT and wq chunk DMAs so the first q-proj accumulation
        # chain (needs the (xts[c], wq_rows[c]) pair per step) streams
        xts, wv_rows = [], []
        for c in range(8):
            t = persist.tile([P, S_HALO], pst, tag=f"xt{c}")
            nc.sync.dma_start(t[:], xT[c * P:(c + 1) * P, :])
            xts.append(t)
            t = persist.tile([P, D], pst, tag=f"wv{c}")
            nc.sync.dma_start(t[:], wvT[c * P:(c + 1) * P, :])
            wv_rows.append(t)
        wq_rows = rowchunks(wqT, "wq", D)
        wk_rows = rowchunks(wkT, "wk", D)
        wo_rows = rowchunks(woT, "wo", D)

        trim_sb = const.tile([P, 1536], ast)
        nc.sync.dma_start(trim_sb[:], trimd[:])

        qT_sb = persist.tile([P, 8, S_OWN], ast)
        kT_sb = persist.tile([P, 8, S_HALO], ast)
        v_sb = persist.tile([P, NKT, H * 65], ast)
        # one tile per feature group: exact dependency tracking lets the
        # output projection's early accumulation terms start as soon as that
        # group's heads are normalized (a single tile made every O matmul
        # wait on the LAST head's normalize)
        outT_g = [
            persist.tile([P, S_OWN], pst, tag=f"outT{g}", name=f"outT{g}")
            for g in range(8)
        ]

        # ================= V projection =================
        # Dedicated deep PSUM pool: during the DMA-gated warmup only ~3 V
        # chains could be in flight with the shared 3-buf pool, idling the PE
        # ~50%; 6 bufs keep the PE saturated while x/wv chunks stream in.
        with tc.tile_pool(name="vpsum", bufs=6, space="PSUM") as vpool:
            for t in range(NKT):
                for fh in range(2):
                    ps = vpool.tile([P, S_OWN], F32, tag="vps")
                    for c in range(8):
                        nc.tensor.matmul(
                            ps[:],
                            xts[c][:, t * P:(t + 1) * P],
                            wv_rows[c][:, fh * S_OWN:(fh + 1) * S_OWN],
                            start=(c == 0),
                            stop=(c == 7),
                        )
                    dst = v_sb[:, t, :].rearrange("p (h x) -> p h x", x=65)[
                        :, fh * 8:(fh + 1) * 8, 0:64
                    ]
                    nc.vector.tensor_copy(dst, ps[:].rearrange("p (h d) -> p h d", d=64))
                onescol = v_sb[:, t, :].rearrange("p (h x) -> p h x", x=65)[:, :, 64:65]
                nc.vector.memset(onescol, 1.0)

        # ================= Q/K projections + attention =================
        ctx2 = ExitStack()
        ppool = ctx2.enter_context(tc.tile_pool(name="qkpsum", bufs=qk_bufs, space="PSUM"))
        if True:
            # q/k per g with both heads of that g emitted right after:
            # attention streams behind the projections on every engine
            epool = ctx2.enter_context(tc.tile_pool(name="esb", bufs=e_bufs))
            rpool = ctx2.enter_context(tc.tile_pool(name="rsb", bufs=4))
            psS_pool = ctx2.enter_context(
                tc.tile_pool(name="psS", bufs=psS_bufs, space="PSUM"))
            psO_pool = ctx2.enter_context(
                tc.tile_pool(name="psO", bufs=psO_bufs, space="PSUM"))

            def project_g(rows, dst_sb, col_lo, col_n, g):
                n_grp = (col_n + S_OWN - 1) // S_OWN
                for f in range(n_grp):
                    n0 = f * S_OWN
                    n1 = min(col_n, n0 + S_OWN)
                    ps = ppool.tile([P, S_OWN], F32, tag="qkps")
                    for c in range(8):
                        nc.tensor.matmul(
                            ps[:, : n1 - n0],
                            rows[c][:, g * P:(g + 1) * P],
                            xts[c][:, col_lo + n0: col_lo + n1],
                            start=(c == 0),
                            stop=(c == 7),
                        )
                    nc.vector.tensor_copy(dst_sb[:, g, n0:n1], ps[:, : n1 - n0])

            # score tiles packed in pairs sharing one PSUM bank: 4 exp ops
            # per head instead of 6 (less Act time + shorter e-ready chain)
            KT_PAIRS = [(0, 1), (2,), (3,), (4, 5)]

            def attend(h):
                g, r0 = h // 2, 64 * (h % 2)
                e_sb = epool.tile([P, 1536], ast, tag="e")
                for pair in KT_PAIRS:
                    psS = psS_pool.tile([P, 384], F32, tag="s")
                    loc = 0
                    for kt in pair:
                        lo, hi = KT_SPANS[kt]
                        N = KT_N[kt]
                        nc.tensor.matmul(
                            psS[:, loc:loc + N],
                            kT_sb[r0:r0 + 64, g, kt * P:(kt + 1) * P],
                            qT_sb[r0:r0 + 64, g, lo * P:(hi + 1) * P],
                            start=True,
                            stop=True,
                            skip_group_check=True,
                        )
                        loc += N
                    off = KT_OFF[pair[0]]
                    nc.scalar.activation(
                        e_sb[:, off:off + loc], psS[:, :loc],
                        mybir.ActivationFunctionType.Exp, scale=0.125,
                    )
                # triangles + sequence-edge validity in one multiplicative pass
                nc.vector.tensor_mul(e_sb[:], e_sb[:], trim_sb[:])

                psO = psO_pool.tile([65, S_OWN], F32, tag="o")
                for i, kt in enumerate([2, 0, 3, 5, 1, 4]):
                    lo, hi = KT_SPANS[kt]
                    off = KT_OFF[kt]
                    nc.tensor.matmul(
                        psO[:, lo * P:(hi + 1) * P],
                        v_sb[:, kt, 65 * h:65 * h + 65],
                        e_sb[:, off:off + KT_N[kt]],
                        start=(i == 0),
                        stop=(i == NKT - 1),
                        skip_group_check=True,
                    )
                recip = rpool.tile([1, S_OWN], F32, tag="r")
                nc.vector.reciprocal(recip[:], psO[64:65, :])
                bsb = rpool.tile([64, S_OWN], F32, tag="bs")
                nc.gpsimd.partition_broadcast(bsb[:], recip[:])
                nc.vector.tensor_mul(outT_g[g][r0:r0 + 64, :], psO[0:64, :], bsb[:])

            for g in range(8):
                project_g(wq_rows, qT_sb, HALO, S_OWN, g)
                project_g(wk_rows, kT_sb, 0, S_HALO, g)
                attend(2 * g)
                attend(2 * g + 1)

            # ================= output projection =================
            # psY reuses the qk projection ring (tag qkps): its first bank was
            # last drained by g7's k-proj, long before the final head's PV, so
            # the first O chain starts immediately after the last attention
            # matmul instead of waiting ~2.3us for a psO drain.
            with tc.tile_pool(name="ysb", bufs=4) as ypool:
                half = S_OWN // 2
                qtr = S_OWN // 4
                for m in range(8):
                    psY = ppool.tile([P, S_OWN], F32, tag="qkps")
                    for c in range(8):
                        nc.tensor.matmul(
                            psY[:],
                            wo_rows[c][:, m * P:(m + 1) * P],
                            outT_g[c][:],
                            start=(c == 0),
                            stop=(c == 7),
                        )
                    y_sb = ypool.tile([P, S_OWN], F32, tag="ysb")
                    ydst = yT.rearrange("(m p) n -> m p n", p=P)[m]
                    if m < 7:
                        # alternate drain engines + split the store so the
                        # output DMA tail overlaps the last matmuls
                        if m % 2 == 0:
                            nc.scalar.copy(y_sb[:, :half], psY[:, :half])
                            nc.scalar.copy(y_sb[:, half:], psY[:, half:])
                        else:
                            nc.vector.tensor_copy(y_sb[:, :half], psY[:, :half])
                            nc.vector.tensor_copy(y_sb[:, half:], psY[:, half:])
                        nc.sync.dma_start(ydst[:, :half], y_sb[:, :half])
                        nc.sync.dma_start(ydst[:, half:], y_sb[:, half:])
                    else:
                        # last chain: drain in quarters on alternating engines
                        # so the drain+DMA pipeline right after the final
                        # matmul is as short as possible
                        for q in range(4):
                            sl = slice(q * qtr, (q + 1) * qtr)
                            if q % 2 == 0:
                                nc.vector.tensor_copy(y_sb[:, sl], psY[:, sl])
                            else:
                                nc.scalar.copy(y_sb[:, sl], psY[:, sl])
                            nc.sync.dma_start(ydst[:, sl], y_sb[:, sl])

        ctx2.close()

    nc.compile()
    return nc


def make_trimask(qs: int) -> np.ndarray:
    m = np.zeros((P, 1536), dtype=np.float32)
    for kt in range(NKT):
        lo, hi = KT_SPANS[kt]
        N = KT_N[kt]
        off = KT_OFF[kt]
        kj = np.arange(P)[:, None]
        f = np.arange(N)[None, :]
        qt = lo + f // P
        qi = f % P
        qg = qs + 128 * qt + qi
        kg = qs - 256 + 128 * kt + kj
        allowed = (kg >= 0) & (kg <= qg) & (kg >= qg - 255)
        m[:, off:off + N] = allowed.astype(np.float32)
    return m


def make_in_maps(x, wq, wk, wv, wo, proj_dt="bf16", att_dt="bf16"):
    npdt = _NP[proj_dt]
    andt = _NP[att_dt]
    wqT = np.ascontiguousarray(wq.T).astype(npdt)
    wkT = np.ascontiguousarray(wk.T).astype(npdt)
    wvT = np.ascontiguousarray(wv.T).astype(npdt)
    woT = np.ascontiguousarray(wo.T).astype(npdt)
    tri_edge = make_trimask(0).astype(andt)
    tri_mid = make_trimask(512).astype(andt)
    in_maps = []
    for c in range(8):
        b, qtr = c // 4, c % 4
        qs = 512 * qtr
        xh = np.zeros((S_HALO, D), dtype=np.float32)
        lo = qs - HALO
        xh[max(0, -lo):, :] = x[b, max(lo, 0): qs + S_OWN, :]
        in_maps.append({
            "xT": np.ascontiguousarray(xh.T).astype(npdt),
            "wqT": wqT, "wkT": wkT, "wvT": wvT, "woT": woT,
            "trimask7": tri_edge if qs == 0 else tri_mid,
        })
    return in_maps


def assemble_output(results):
    y = np.empty((2, 2048, D), dtype=np.float32)
    for c in range(8):
        b, qtr = c // 4, c % 4
        qs = 512 * qtr
        y[b, qs:qs + S_OWN, :] = results[c]["yT"].T
    return y


_NC_CACHE = {}
B, S = 2, 2048


def kernel(x, wq, wk, wv, wo):
    from concourse.bass_utils import run_bass_kernel_spmd
    x = np.asarray(x, dtype=np.float32)
    assert x.shape == (B, S, D), f"unexpected x shape {x.shape}"
    if "nc" not in _NC_CACHE:
        _NC_CACHE["nc"] = build_nc(psS_bufs=3, psO_bufs=2, qk_bufs=3)
    nc = _NC_CACHE["nc"]
    in_maps = make_in_maps(
        x, np.asarray(wq, np.float32), np.asarray(wk, np.float32),
        np.asarray(wv, np.float32), np.asarray(wo, np.float32))
    res = run_bass_kernel_spmd(nc, in_maps, core_ids=list(range(8)))
    return assemble_output(res.results)



# revision 10
# speedup vs baseline: 1.0014x; 1.0014x over previous
"""Sliding-window attention (B=2, S=2048, D=1024, H=16, window=256) on 8
Trainium2 NeuronCores.

Sharding: 8 cores = 2 batches x 4 sequence quarters (512 queries each).
The 256-wide causal window makes attention local, so each core carries a
256-position left halo of x for K/V and produces a clean output shard --
no collectives and no cross-core reduction.

Per-core device kernel (transposed/feature-major layouts, zero on-chip
transposes), fully software-pipelined: V projection first, then per
feature-group g the q/k projections IMMEDIATELY followed by that group's
two attention heads, so scores/exp/mask/PV stream behind the remaining
projections on all five engines. Per head: scoresT[key, query] K=64
matmuls -> exp straight from PSUM on the scalar engine -> one
multiplicative trimask DVE op (triangles + sequence-edge validity,
host-built per core) -> attention x V with the softmax denominator fused
via a ones column in V (no zero-init: the first PV matmul's start=True
bank-clear + overwrite-on-clear handles initialization, HW-verified
semantics) -> reciprocal + gpsimd partition broadcast.
bf16 operands, fp32 PSUM accumulation; weights stream as contiguous
[128,1024] row chunks issued up front. HW-validated rel L2 err ~5e-3.
"""
from contextlib import ExitStack

import numpy as np
import ml_dtypes

import concourse.bass as bass
import concourse.mybir as mybir
import concourse.tile as tile
from concourse import bacc

F32 = mybir.dt.float32
BF16 = mybir.dt.bfloat16

D = 1024
H = 16
S_OWN = 512
HALO = 256
S_HALO = 768
NQT = 4
NKT = 6
P = 128

KT_SPANS = [(max(0, kt - 2), min(NQT - 1, kt)) for kt in range(NKT)]
KT_N = [128 * (hi - lo + 1) for (lo, hi) in KT_SPANS]
KT_OFF = np.concatenate([[0], np.cumsum(KT_N)]).tolist()

_STORE = {"f32": F32, "bf16": BF16}
_NP = {"f32": np.float32, "bf16": ml_dtypes.bfloat16}


def build_nc(proj_dt="bf16", att_dt="bf16", psS_bufs=4, psO_bufs=2, e_bufs=4, qk_bufs=2):
    pst = _STORE[proj_dt]
    ast = _STORE[att_dt]

    nc = bacc.Bacc("TRN2", target_bir_lowering=False, debug=False, num_devices=8)

    xT = nc.dram_tensor("xT", [D, S_HALO], pst, kind="ExternalInput").ap()
    wqT = nc.dram_tensor("wqT", [D, D], pst, kind="ExternalInput").ap()
    wkT = nc.dram_tensor("wkT", [D, D], pst, kind="ExternalInput").ap()
    wvT = nc.dram_tensor("wvT", [D, D], pst, kind="ExternalInput").ap()
    woT = nc.dram_tensor("woT", [D, D], pst, kind="ExternalInput").ap()
    trimd = nc.dram_tensor("trimask7", [P, 1536], ast, kind="ExternalInput").ap()
    yT = nc.dram_tensor("yT", [D, S_OWN], F32, kind="ExternalOutput").ap()

    with nc.allow_low_precision(reason="bf16 compute, f32 psum accumulation"), \
         tile.TileContext(nc) as tc, ExitStack() as ctx:
        const = ctx.enter_context(tc.tile_pool(name="const", bufs=1))
        persist = ctx.enter_context(tc.tile_pool(name="persist", bufs=1))

        def rowchunks(wT, name, width):
            ts_ = []
            for c in range(8):
                t = persist.tile([P, width], pst, tag=f"{name}{c}")
                nc.sync.dma_start(t[:], wT[c * P:(c + 1) * P, :])
                ts_.append(t)
            return ts_

        # interleave xT and wq chunk DMAs so the first q-proj accumulation
        # chain (needs the (xts[c], wq_rows[c]) pair per step) streams
        xts, wv_rows = [], []
        for c in range(8):
            t = persist.tile([P, S_HALO], pst, tag=f"xt{c}")
            nc.sync.dma_start(t[:], xT[c * P:(c + 1) * P, :])
            xts.append(t)
            t = persist.tile([P, D], pst, tag=f"wv{c}")
            nc.sync.dma_start(t[:], wvT[c * P:(c + 1) * P, :])
            wv_rows.append(t)
        wq_rows = rowchunks(wqT, "wq", D)
        wk_rows = rowchunks(wkT, "wk", D)
        wo_rows = rowchunks(woT, "wo", D)

        trim_sb = const.tile([P, 1536], ast)
        nc.sync.dma_start(trim_sb[:], trimd[:])

        qT_sb = persist.tile([P, 8, S_OWN], ast)
        kT_sb = persist.tile([P, 8, S_HALO], ast)
        v_sb = persist.tile([P, NKT, H * 65], ast)
        # one tile per feature group: exact dependency tracking lets the
        # output projection's early accumulation terms start as soon as that
        # group's heads are normalized (a single tile made every O matmul
        # wait on the LAST head's normalize)
        outT_g = [
            persist.tile([P, S_OWN], pst, tag=f"outT{g}", name=f"outT{g}")
            for g in range(8)
        ]

        # ================= V projection =================
        # Dedicated deep PSUM pool: during the DMA-gated warmup only ~3 V
        # chains could be in flight with the shared 3-buf pool, idling the PE
        # ~50%; 6 bufs keep the PE saturated while x/wv chunks stream in.
        with tc.tile_pool(name="vpsum", bufs=6, space="PSUM") as vpool:
            for t in range(NKT):
                for fh in range(2):
                    ps = vpool.tile([P, S_OWN], F32, tag="vps")
                    for c in range(8):
                        nc.tensor.matmul(
                            ps[:],
                            xts[c][:, t * P:(t + 1) * P],
                            wv_rows[c][:, fh * S_OWN:(fh + 1) * S_OWN],
                            start=(c == 0),
                            stop=(c == 7),
                        )
                    dst = v_sb[:, t, :].rearrange("p (h x) -> p h x", x=65)[
                        :, fh * 8:(fh + 1) * 8, 0:64
                    ]
                    nc.vector.tensor_copy(dst, ps[:].rearrange("p (h d) -> p h d", d=64))
                onescol = v_sb[:, t, :].rearrange("p (h x) -> p h x", x=65)[:, :, 64:65]
                nc.vector.memset(onescol, 1.0)

        # ================= Q/K projections + attention =================
        ctx2 = ExitStack()
        ppool = ctx2.enter_context(tc.tile_pool(name="qkpsum", bufs=qk_bufs, space="PSUM"))
        if True:
            # q/k per g with both heads of that g emitted right after:
            # attention streams behind the projections on every engine
            epool = ctx2.enter_context(tc.tile_pool(name="esb", bufs=e_bufs))
            rpool = ctx2.enter_context(tc.tile_pool(name="rsb", bufs=4))
            psS_pool = ctx2.enter_context(
                tc.tile_pool(name="psS", bufs=psS_bufs, space="PSUM"))
            psO_pool = ctx2.enter_context(
                tc.tile_pool(name="psO", bufs=psO_bufs, space="PSUM"))

            def project_g(rows, dst_sb, col_lo, col_n, g):
                n_grp = (col_n + S_OWN - 1) // S_OWN
                for f in range(n_grp):
                    n0 = f * S_OWN
                    n1 = min(col_n, n0 + S_OWN)
                    ps = ppool.tile([P, S_OWN], F32, tag="qkps")
                    for c in range(8):
                        nc.tensor.matmul(
                            ps[:, : n1 - n0],
                            rows[c][:, g * P:(g + 1) * P],
                            xts[c][:, col_lo + n0: col_lo + n1],
                            start=(c == 0),
                            stop=(c == 7),
                        )
                    nc.vector.tensor_copy(dst_sb[:, g, n0:n1], ps[:, : n1 - n0])

            # score tiles packed in pairs sharing one PSUM bank: 4 exp ops
            # per head instead of 6 (less Act time + shorter e-ready chain)
            KT_PAIRS = [(0, 1), (2,), (3,), (4, 5)]

            def attend(h):
                g, r0 = h // 2, 64 * (h % 2)
                e_sb = epool.tile([P, 1536], ast, tag="e")
                for pair in KT_PAIRS:
                    psS = psS_pool.tile([P, 384], F32, tag="s")
                    loc = 0
                    for kt in pair:
                        lo, hi = KT_SPANS[kt]
                        N = KT_N[kt]
                        nc.tensor.matmul(
                            psS[:, loc:loc + N],
                            kT_sb[r0:r0 + 64, g, kt * P:(kt + 1) * P],
                            qT_sb[r0:r0 + 64, g, lo * P:(hi + 1) * P],
                            start=True,
                            stop=True,
                            skip_group_check=True,
                        )
                        loc += N
                    off = KT_OFF[pair[0]]
                    nc.scalar.activation(
                        e_sb[:, off:off + loc], psS[:, :loc],
                        mybir.ActivationFunctionType.Exp, scale=0.125,
                    )
                # triangles + sequence-edge validity in one multiplicative pass
                nc.vector.tensor_mul(e_sb[:], e_sb[:], trim_sb[:])

                psO = psO_pool.tile([65, S_OWN], F32, tag="o")
                for i, kt in enumerate([2, 0, 3, 5, 1, 4]):
                    lo, hi = KT_SPANS[kt]
                    off = KT_OFF[kt]
                    nc.tensor.matmul(
                        psO[:, lo * P:(hi + 1) * P],
                        v_sb[:, kt, 65 * h:65 * h + 65],
                        e_sb[:, off:off + KT_N[kt]],
                        start=(i == 0),
                        stop=(i == NKT - 1),
                        skip_group_check=True,
                    )
                recip = rpool.tile([1, S_OWN], F32, tag="r")
                nc.vector.reciprocal(recip[:], psO[64:65, :])
                bsb = rpool.tile([64, S_OWN], F32, tag="bs")
                nc.gpsimd.partition_broadcast(bsb[:], recip[:])
                nc.vector.tensor_mul(outT_g[g][r0:r0 + 64, :], psO[0:64, :], bsb[:])

            for g in range(8):
                project_g(wq_rows, qT_sb, HALO, S_OWN, g)
                project_g(wk_rows, kT_sb, 0, S_HALO, g)
                attend(2 * g)
                attend(2 * g + 1)

            # ================= output projection =================
            # psY reuses the qk projection ring (tag qkps): its first bank was
            # last drained by g7's k-proj, long before the final head's PV, so
            # the first O chain starts immediately after the last attention
            # matmul instead of waiting ~2.3us for a psO drain.
            with tc.tile_pool(name="ysb", bufs=4) as ypool:
                half = S_OWN // 2
                qtr = S_OWN // 4
                for m in range(8):
                    psY = ppool.tile([P, S_OWN], F32, tag="qkps")
                    for c in range(8):
                        nc.tensor.matmul(
                            psY[:],
                            wo_rows[c][:, m * P:(m + 1) * P],
                            outT_g[c][:],
                            start=(c == 0),
                            stop=(c == 7),
                        )
                    y_sb = ypool.tile([P, S_OWN], F32, tag="ysb")
                    ydst = yT.rearrange("(m p) n -> m p n", p=P)[m]
                    # alternate drain engines + split the store so the output
                    # DMA tail overlaps the last matmuls
                    if m % 2 == 0:
                        nc.scalar.copy(y_sb[:, :half], psY[:, :half])
                        nc.scalar.copy(y_sb[:, half:], psY[:, half:])
                    else:
                        nc.vector.tensor_copy(y_sb[:, :half], psY[:, :half])
                        nc.vector.tensor_copy(y_sb[:, half:], psY[:, half:])
                    nc.sync.dma_start(ydst[:, :half], y_sb[:, :half])
                    nc.sync.dma_start(ydst[:, half:], y_sb[:, half:])

        ctx2.close()

    nc.compile()
    return nc


def make_trimask(qs: int) -> np.ndarray:
    m = np.zeros((P, 1536), dtype=np.float32)
    for kt in range(NKT):
        lo, hi = KT_SPANS[kt]
        N = KT_N[kt]
        off = KT_OFF[kt]
        kj = np.arange(P)[:, None]
        f = np.arange(N)[None, :]
        qt = lo + f // P
        qi = f % P
        qg = qs + 128 * qt + qi
        kg = qs - 256 + 128 * kt + kj
        allowed = (kg >= 0) & (kg <= qg) & (kg >= qg - 255)
        m[:, off:off + N] = allowed.astype(np.float32)
    return m


def make_in_maps(x, wq, wk, wv, wo, proj_dt="bf16", att_dt="bf16"):
    npdt = _NP[proj_dt]
    andt = _NP[att_dt]
    wqT = np.ascontiguousarray(wq.T).astype(npdt)
    wkT = np.ascontiguousarray(wk.T).astype(npdt)
    wvT = np.ascontiguousarray(wv.T).astype(npdt)
    woT = np.ascontiguousarray(wo.T).astype(npdt)
    tri_edge = make_trimask(0).astype(andt)
    tri_mid = make_trimask(512).astype(andt)
    in_maps = []
    for c in range(8):
        b, qtr = c // 4, c % 4
        qs = 512 * qtr
        xh = np.zeros((S_HALO, D), dtype=np.float32)
        lo = qs - HALO
        xh[max(0, -lo):, :] = x[b, max(lo, 0): qs + S_OWN, :]
        in_maps.append({
            "xT": np.ascontiguousarray(xh.T).astype(npdt),
            "wqT": wqT, "wkT": wkT, "wvT": wvT, "woT": woT,
            "trimask7": tri_edge if qs == 0 else tri_mid,
        })
    return in_maps


def assemble_output(results):
    y = np.empty((2, 2048, D), dtype=np.float32)
    for c in range(8):
        b, qtr = c // 4, c % 4
        qs = 512 * qtr
        y[b, qs:qs + S_OWN, :] = results[c]["yT"].T
    return y


_NC_CACHE = {}
B, S = 2, 2048


def kernel(x, wq, wk, wv, wo):
    from concourse.bass_utils import run_bass_kernel_spmd
    x = np.asarray(x, dtype=np.float32)
    assert x.shape == (B, S, D), f"unexpected x shape {x.shape}"
    if "nc" not in _NC_CACHE:
        _NC_CACHE["nc"] = build_nc(psS_bufs=2, psO_bufs=2, qk_bufs=4)
    nc = _NC_CACHE["nc"]
    in_maps = make_in_maps(
        x, np.asarray(wq, np.float32), np.asarray(wk, np.float32),
        np.asarray(wv, np.float32), np.asarray(wo, np.float32))
    res = run_bass_kernel_spmd(nc, in_maps, core_ids=list(range(8)))
    return assemble_output(res.results)



# revision 13
# speedup vs baseline: 1.0035x; 1.0020x over previous
"""Sliding-window attention (B=2, S=2048, D=1024, H=16, window=256) on 8
Trainium2 NeuronCores.

Sharding: 8 cores = 2 batches x 4 sequence quarters (512 queries each).
The 256-wide causal window makes attention local, so each core carries a
256-position left halo of x for K/V and produces a clean output shard --
no collectives and no cross-core reduction.

Per-core device kernel (transposed/feature-major layouts, zero on-chip
transposes), fully software-pipelined: V projection first, then per
feature-group g the q/k projections IMMEDIATELY followed by that group's
two attention heads, so scores/exp/mask/PV stream behind the remaining
projections on all five engines. Per head: scoresT[key, query] K=64
matmuls -> exp straight from PSUM on the scalar engine -> one
multiplicative trimask DVE op (triangles + sequence-edge validity,
host-built per core) -> attention x V with the softmax denominator fused
via a ones column in V (no zero-init: the first PV matmul's start=True
bank-clear + overwrite-on-clear handles initialization, HW-verified
semantics) -> reciprocal + gpsimd partition broadcast.
bf16 operands, fp32 PSUM accumulation; weights stream as contiguous
[128,1024] row chunks issued up front. HW-validated rel L2 err ~5e-3.
"""
from contextlib import ExitStack

import numpy as np
import ml_dtypes

import concourse.bass as bass
import concourse.mybir as mybir
import concourse.tile as tile
from concourse import bacc

F32 = mybir.dt.float32
BF16 = mybir.dt.bfloat16

D = 1024
H = 16
S_OWN = 512
HALO = 256
S_HALO = 768
NQT = 4
NKT = 6
P = 128

KT_SPANS = [(max(0, kt - 2), min(NQT - 1, kt)) for kt in range(NKT)]
KT_N = [128 * (hi - lo + 1) for (lo, hi) in KT_SPANS]
KT_OFF = np.concatenate([[0], np.cumsum(KT_N)]).tolist()

_STORE = {"f32": F32, "bf16": BF16}
_NP = {"f32": np.float32, "bf16": ml_dtypes.bfloat16}


def build_nc(proj_dt="bf16", att_dt="bf16", psS_bufs=4, psO_bufs=2, e_bufs=4, qk_bufs=2):
    pst = _STORE[proj_dt]
    ast = _STORE[att_dt]

    nc = bacc.Bacc("TRN2", target_bir_lowering=False, debug=False, num_devices=8)

    xT = nc.dram_tensor("xT", [D, S_HALO], pst, kind="ExternalInput").ap()
    wqT = nc.dram_tensor("wqT", [D, D], pst, kind="ExternalInput").ap()
    wkT = nc.dram_tensor("wkT", [D, D], pst, kind="ExternalInput").ap()
    wvT = nc.dram_tensor("wvT", [D, D], pst, kind="ExternalInput").ap()
    woT = nc.dram_tensor("woT", [D, D], pst, kind="ExternalInput").ap()
    trimd = nc.dram_tensor("trimask7", [P, 1536], ast, kind="ExternalInput").ap()
    yT = nc.dram_tensor("yT", [D, S_OWN], F32, kind="ExternalOutput").ap()

    with nc.allow_low_precision(reason="bf16 compute, f32 psum accumulation"), \
         tile.TileContext(nc) as tc, ExitStack() as ctx:
        const = ctx.enter_context(tc.tile_pool(name="const", bufs=1))
        persist = ctx.enter_context(tc.tile_pool(name="persist", bufs=1))

        def rowchunks(wT, name, width):
            ts_ = []
            for c in range(8):
                t = persist.tile([P, width], pst, tag=f"{name}{c}")
                nc.sync.dma_start(t[:], wT[c * P:(c + 1) * P, :])
                ts_.append(t)
            return ts_

        # interleave xT and wq chunk DMAs so the first q-proj accumulation
        # chain (needs the (xts[c], wq_rows[c]) pair per step) streams
        xts, wv_rows = [], []
        for c in range(8):
            t = persist.tile([P, S_HALO], pst, tag=f"xt{c}")
            nc.sync.dma_start(t[:], xT[c * P:(c + 1) * P, :])
            xts.append(t)
            t = persist.tile([P, D], pst, tag=f"wv{c}")
            nc.sync.dma_start(t[:], wvT[c * P:(c + 1) * P, :])
            wv_rows.append(t)
        wq_rows = rowchunks(wqT, "wq", D)
        wk_rows = rowchunks(wkT, "wk", D)
        wo_rows = rowchunks(woT, "wo", D)

        trim_sb = const.tile([P, 1536], ast)
        nc.sync.dma_start(trim_sb[:], trimd[:])

        qT_sb = persist.tile([P, 8, S_OWN], ast)
        kT_sb = persist.tile([P, 8, S_HALO], ast)
        v_sb = persist.tile([P, NKT, H * 65], ast)
        # one tile per feature group: exact dependency tracking lets the
        # output projection's early accumulation terms start as soon as that
        # group's heads are normalized (a single tile made every O matmul
        # wait on the LAST head's normalize)
        outT_g = [
            persist.tile([P, S_OWN], pst, tag=f"outT{g}", name=f"outT{g}")
            for g in range(8)
        ]

        # ================= V projection =================
        # Dedicated deep PSUM pool: during the DMA-gated warmup only ~3 V
        # chains could be in flight with the shared 3-buf pool, idling the PE
        # ~50%; 6 bufs keep the PE saturated while x/wv chunks stream in.
        with tc.tile_pool(name="vpsum", bufs=6, space="PSUM") as vpool:
            for t in range(NKT):
                for fh in range(2):
                    ps = vpool.tile([P, S_OWN], F32, tag="vps")
                    for c in range(8):
                        nc.tensor.matmul(
                            ps[:],
                            xts[c][:, t * P:(t + 1) * P],
                            wv_rows[c][:, fh * S_OWN:(fh + 1) * S_OWN],
                            start=(c == 0),
                            stop=(c == 7),
                        )
                    dst = v_sb[:, t, :].rearrange("p (h x) -> p h x", x=65)[
                        :, fh * 8:(fh + 1) * 8, 0:64
                    ]
                    nc.vector.tensor_copy(dst, ps[:].rearrange("p (h d) -> p h d", d=64))
                onescol = v_sb[:, t, :].rearrange("p (h x) -> p h x", x=65)[:, :, 64:65]
                nc.vector.memset(onescol, 1.0)

        # ================= Q/K projections + attention =================
        ctx2 = ExitStack()
        ppool = ctx2.enter_context(tc.tile_pool(name="qkpsum", bufs=qk_bufs, space="PSUM"))
        if True:
            # q/k per g with both heads of that g emitted right after:
            # attention streams behind the projections on every engine
            epool = ctx2.enter_context(tc.tile_pool(name="esb", bufs=e_bufs))
            rpool = ctx2.enter_context(tc.tile_pool(name="rsb", bufs=4))
            psS_pool = ctx2.enter_context(
                tc.tile_pool(name="psS", bufs=psS_bufs, space="PSUM"))
            psO_pool = ctx2.enter_context(
                tc.tile_pool(name="psO", bufs=psO_bufs, space="PSUM"))

            def project_g(rows, dst_sb, col_lo, col_n, g):
                n_grp = (col_n + S_OWN - 1) // S_OWN
                for f in range(n_grp):
                    n0 = f * S_OWN
                    n1 = min(col_n, n0 + S_OWN)
                    ps = ppool.tile([P, S_OWN], F32, tag="qkps")
                    for c in range(8):
                        nc.tensor.matmul(
                            ps[:, : n1 - n0],
                            rows[c][:, g * P:(g + 1) * P],
                            xts[c][:, col_lo + n0: col_lo + n1],
                            start=(c == 0),
                            stop=(c == 7),
                        )
                    nc.vector.tensor_copy(dst_sb[:, g, n0:n1], ps[:, : n1 - n0])

            # score tiles packed in pairs sharing one PSUM bank: 4 exp ops
            # per head instead of 6 (less Act time + shorter e-ready chain)
            KT_PAIRS = [(0, 1), (2,), (3,), (4, 5)]

            def attend(h):
                g, r0 = h // 2, 64 * (h % 2)
                e_sb = epool.tile([P, 1536], ast, tag="e")
                for pair in KT_PAIRS:
                    psS = psS_pool.tile([P, 384], F32, tag="s")
                    loc = 0
                    for kt in pair:
                        lo, hi = KT_SPANS[kt]
                        N = KT_N[kt]
                        nc.tensor.matmul(
                            psS[:, loc:loc + N],
                            kT_sb[r0:r0 + 64, g, kt * P:(kt + 1) * P],
                            qT_sb[r0:r0 + 64, g, lo * P:(hi + 1) * P],
                            start=True,
                            stop=True,
                            skip_group_check=True,
                        )
                        loc += N
                    off = KT_OFF[pair[0]]
                    nc.scalar.activation(
                        e_sb[:, off:off + loc], psS[:, :loc],
                        mybir.ActivationFunctionType.Exp, scale=0.125,
                    )
                # triangles + sequence-edge validity in one multiplicative pass
                nc.vector.tensor_mul(e_sb[:], e_sb[:], trim_sb[:])

                psO = psO_pool.tile([65, S_OWN], F32, tag="o")
                for i, kt in enumerate([2, 0, 3, 5, 1, 4]):
                    lo, hi = KT_SPANS[kt]
                    off = KT_OFF[kt]
                    nc.tensor.matmul(
                        psO[:, lo * P:(hi + 1) * P],
                        v_sb[:, kt, 65 * h:65 * h + 65],
                        e_sb[:, off:off + KT_N[kt]],
                        start=(i == 0),
                        stop=(i == NKT - 1),
                        skip_group_check=True,
                    )
                recip = rpool.tile([1, S_OWN], F32, tag="r")
                nc.vector.reciprocal(recip[:], psO[64:65, :])
                bsb = rpool.tile([64, S_OWN], F32, tag="bs")
                nc.gpsimd.partition_broadcast(bsb[:], recip[:])
                nc.vector.tensor_mul(outT_g[g][r0:r0 + 64, :], psO[0:64, :], bsb[:])

            for g in range(8):
                project_g(wq_rows, qT_sb, HALO, S_OWN, g)
                project_g(wk_rows, kT_sb, 0, S_HALO, g)
                attend(2 * g)
                attend(2 * g + 1)

            # ================= output projection =================
            # psY reuses the qk projection ring (tag qkps): its first bank was
            # last drained by g7's k-proj, long before the final head's PV, so
            # the first O chain starts immediately after the last attention
            # matmul instead of waiting ~2.3us for a psO drain.
            # Two-pass output projection: pass 1 (groups 0-3) is runnable as
            # soon as heads 0-7 are normalized, so those chains flow through
            # the PSUM ring during the back half of attention and fill the
            # dead window while the final head's softmax normalize completes.
            # Pass 2 (groups 4-7) accumulates the rest; its drain is a DVE
            # add onto the pass-1 partial (no extra op on the tail).
            with tc.tile_pool(name="ysb", bufs=1) as ypool:
                half = S_OWN // 2
                y_acc = []
                for m in range(8):
                    psY = ppool.tile([P, S_OWN], F32, tag="qkps")
                    for c in range(4):
                        nc.tensor.matmul(
                            psY[:],
                            wo_rows[c][:, m * P:(m + 1) * P],
                            outT_g[c][:],
                            start=(c == 0),
                            stop=(c == 3),
                        )
                    ya = ypool.tile([P, S_OWN], F32, tag=f"yacc{m}", name=f"ya{m}")
                    if m % 2 == 0:
                        nc.scalar.copy(ya[:, :half], psY[:, :half])
                        nc.scalar.copy(ya[:, half:], psY[:, half:])
                    else:
                        nc.vector.tensor_copy(ya[:], psY[:])
                    y_acc.append(ya)
                for m in range(8):
                    psY = ppool.tile([P, S_OWN], F32, tag="qkps")
                    for c in range(4, 8):
                        nc.tensor.matmul(
                            psY[:],
                            wo_rows[c][:, m * P:(m + 1) * P],
                            outT_g[c][:],
                            start=(c == 4),
                            stop=(c == 7),
                        )
                    y_sb = ypool.tile([P, S_OWN], F32, tag="ysb", bufs=4)
                    nc.vector.tensor_add(y_sb[:], y_acc[m][:], psY[:])
                    ydst = yT.rearrange("(m p) n -> m p n", p=P)[m]
                    nc.sync.dma_start(ydst[:, :half], y_sb[:, :half])
                    nc.sync.dma_start(ydst[:, half:], y_sb[:, half:])

        ctx2.close()

    nc.compile()
    return nc


def make_trimask(qs: int) -> np.ndarray:
    m = np.zeros((P, 1536), dtype=np.float32)
    for kt in range(NKT):
        lo, hi = KT_SPANS[kt]
        N = KT_N[kt]
        off = KT_OFF[kt]
        kj = np.arange(P)[:, None]
        f = np.arange(N)[None, :]
        qt = lo + f // P
        qi = f % P
        qg = qs + 128 * qt + qi
        kg = qs - 256 + 128 * kt + kj
        allowed = (kg >= 0) & (kg <= qg) & (kg >= qg - 255)
        m[:, off:off + N] = allowed.astype(np.float32)
    return m


def make_in_maps(x, wq, wk, wv, wo, proj_dt="bf16", att_dt="bf16"):
    npdt = _NP[proj_dt]
    andt = _NP[att_dt]
    wqT = np.ascontiguousarray(wq.T).astype(npdt)
    wkT = np.ascontiguousarray(wk.T).astype(npdt)
    wvT = np.ascontiguousarray(wv.T).astype(npdt)
    woT = np.ascontiguousarray(wo.T).astype(npdt)
    tri_edge = make_trimask(0).astype(andt)
    tri_mid = make_trimask(512).astype(andt)
    in_maps = []
    for c in range(8):
        b, qtr = c // 4, c % 4
        qs = 512 * qtr
        xh = np.zeros((S_HALO, D), dtype=np.float32)
        lo = qs - HALO
        xh[max(0, -lo):, :] = x[b, max(lo, 0): qs + S_OWN, :]
        in_maps.append({
            "xT": np.ascontiguousarray(xh.T).astype(npdt),
            "wqT": wqT, "wkT": wkT, "wvT": wvT, "woT": woT,
            "trimask7": tri_edge if qs == 0 else tri_mid,
        })
    return in_maps


def assemble_output(results):
    y = np.empty((2, 2048, D), dtype=np.float32)
    for c in range(8):
        b, qtr = c // 4, c % 4
        qs = 512 * qtr
        y[b, qs:qs + S_OWN, :] = results[c]["yT"].T
    return y


_NC_CACHE = {}
B, S = 2, 2048


def kernel(x, wq, wk, wv, wo):
    from concourse.bass_utils import run_bass_kernel_spmd
    x = np.asarray(x, dtype=np.float32)
    assert x.shape == (B, S, D), f"unexpected x shape {x.shape}"
    if "nc" not in _NC_CACHE:
        _NC_CACHE["nc"] = build_nc(psS_bufs=3, psO_bufs=2, qk_bufs=3)
    nc = _NC_CACHE["nc"]
    in_maps = make_in_maps(
        x, np.asarray(wq, np.float32), np.asarray(wk, np.float32),
        np.asarray(wv, np.float32), np.asarray(wo, np.float32))
    res = run_bass_kernel_spmd(nc, in_maps, core_ids=list(range(8)))
    return assemble_output(res.results)



# revision 14
# speedup vs baseline: 1.0350x; 1.0315x over previous
"""Sliding-window attention (B=2, S=2048, D=1024, H=16, window=256) on 8
Trainium2 NeuronCores.

Sharding: 8 cores = 2 batches x 4 sequence quarters (512 queries each).
The 256-wide causal window makes attention local, so each core carries a
256-position left halo of x for K/V and produces a clean output shard --
no collectives and no cross-core reduction.

Per-core device kernel (transposed/feature-major layouts, zero on-chip
transposes), fully software-pipelined: V projection first, then per
feature-group g the q/k projections IMMEDIATELY followed by that group's
two attention heads, so scores/exp/mask/PV stream behind the remaining
projections on all five engines. Per head: scoresT[key, query] K=64
matmuls -> exp straight from PSUM on the scalar engine -> one
multiplicative trimask DVE op (triangles + sequence-edge validity,
host-built per core) -> attention x V with the softmax denominator fused
via a ones column in V (no zero-init: the first PV matmul's start=True
bank-clear + overwrite-on-clear handles initialization, HW-verified
semantics) -> reciprocal + gpsimd partition broadcast.
bf16 operands, fp32 PSUM accumulation; weights stream as contiguous
[128,1024] row chunks issued up front. HW-validated rel L2 err ~5e-3.
"""
from contextlib import ExitStack

import numpy as np
import ml_dtypes

import concourse.bass as bass
import concourse.mybir as mybir
import concourse.tile as tile
from concourse import bacc

F32 = mybir.dt.float32
BF16 = mybir.dt.bfloat16

D = 1024
H = 16
S_OWN = 512
HALO = 256
S_HALO = 768
NQT = 4
NKT = 6
P = 128

KT_SPANS = [(max(0, kt - 2), min(NQT - 1, kt)) for kt in range(NKT)]
KT_N = [128 * (hi - lo + 1) for (lo, hi) in KT_SPANS]
KT_OFF = np.concatenate([[0], np.cumsum(KT_N)]).tolist()

_STORE = {"f32": F32, "bf16": BF16}
_NP = {"f32": np.float32, "bf16": ml_dtypes.bfloat16}


def build_nc(proj_dt="bf16", att_dt="bf16", psS_bufs=4, psO_bufs=2, e_bufs=4, qk_bufs=2):
    pst = _STORE[proj_dt]
    ast = _STORE[att_dt]

    nc = bacc.Bacc("TRN2", target_bir_lowering=False, debug=False, num_devices=8)

    xT = nc.dram_tensor("xT", [D, S_HALO], pst, kind="ExternalInput").ap()
    wqT = nc.dram_tensor("wqT", [D, D], pst, kind="ExternalInput").ap()
    wkT = nc.dram_tensor("wkT", [D, D], pst, kind="ExternalInput").ap()
    wvT = nc.dram_tensor("wvT", [D, D], pst, kind="ExternalInput").ap()
    woT = nc.dram_tensor("woT", [D, D], pst, kind="ExternalInput").ap()
    trimd = nc.dram_tensor("trimask7", [P, 1536], ast, kind="ExternalInput").ap()
    yT = nc.dram_tensor("yT", [D, S_OWN], F32, kind="ExternalOutput").ap()

    with nc.allow_low_precision(reason="bf16 compute, f32 psum accumulation"), \
         tile.TileContext(nc) as tc, ExitStack() as ctx:
        const = ctx.enter_context(tc.tile_pool(name="const", bufs=1))
        persist = ctx.enter_context(tc.tile_pool(name="persist", bufs=1))

        def rowchunks(wT, name, width):
            ts_ = []
            for c in range(8):
                t = persist.tile([P, width], pst, tag=f"{name}{c}")
                nc.sync.dma_start(t[:], wT[c * P:(c + 1) * P, :])
                ts_.append(t)
            return ts_

        # interleave xT and wq chunk DMAs so the first q-proj accumulation
        # chain (needs the (xts[c], wq_rows[c]) pair per step) streams
        xts, wv_rows = [], []
        for c in range(8):
            t = persist.tile([P, S_HALO], pst, tag=f"xt{c}")
            nc.sync.dma_start(t[:], xT[c * P:(c + 1) * P, :])
            xts.append(t)
            t = persist.tile([P, D], pst, tag=f"wv{c}")
            nc.sync.dma_start(t[:], wvT[c * P:(c + 1) * P, :])
            wv_rows.append(t)
        wq_rows = rowchunks(wqT, "wq", D)
        wk_rows = rowchunks(wkT, "wk", D)
        wo_rows = rowchunks(woT, "wo", D)

        trim_sb = const.tile([P, 1536], ast)
        nc.sync.dma_start(trim_sb[:], trimd[:])

        qT_sb = persist.tile([P, 8, S_OWN], ast)
        kT_sb = persist.tile([P, 8, S_HALO], ast)
        v_sb = persist.tile([P, NKT, H * 65], ast)
        # one tile per feature group: exact dependency tracking lets the
        # output projection's early accumulation terms start as soon as that
        # group's heads are normalized (a single tile made every O matmul
        # wait on the LAST head's normalize)
        outT_g = [
            persist.tile([P, S_OWN], pst, tag=f"outT{g}", name=f"outT{g}")
            for g in range(8)
        ]

        # ================= V projection =================
        # Dedicated deep PSUM pool: during the DMA-gated warmup only ~3 V
        # chains could be in flight with the shared 3-buf pool, idling the PE
        # ~50%; 6 bufs keep the PE saturated while x/wv chunks stream in.
        with tc.tile_pool(name="vpsum", bufs=6, space="PSUM") as vpool:
            for t in range(NKT):
                for fh in range(2):
                    ps = vpool.tile([P, S_OWN], F32, tag="vps")
                    for c in range(8):
                        nc.tensor.matmul(
                            ps[:],
                            xts[c][:, t * P:(t + 1) * P],
                            wv_rows[c][:, fh * S_OWN:(fh + 1) * S_OWN],
                            start=(c == 0),
                            stop=(c == 7),
                        )
                    dst = v_sb[:, t, :].rearrange("p (h x) -> p h x", x=65)[
                        :, fh * 8:(fh + 1) * 8, 0:64
                    ]
                    nc.vector.tensor_copy(dst, ps[:].rearrange("p (h d) -> p h d", d=64))
                onescol = v_sb[:, t, :].rearrange("p (h x) -> p h x", x=65)[:, :, 64:65]
                nc.vector.memset(onescol, 1.0)

        # ================= Q/K projections + attention =================
        ctx2 = ExitStack()
        ppool = ctx2.enter_context(tc.tile_pool(name="qkpsum", bufs=qk_bufs, space="PSUM"))
        if True:
            # q/k per g with both heads of that g emitted right after:
            # attention streams behind the projections on every engine
            epool = ctx2.enter_context(tc.tile_pool(name="esb", bufs=e_bufs))
            rpool = ctx2.enter_context(tc.tile_pool(name="rsb", bufs=4))
            psS_pool = ctx2.enter_context(
                tc.tile_pool(name="psS", bufs=psS_bufs, space="PSUM"))
            psO_pool = ctx2.enter_context(
                tc.tile_pool(name="psO", bufs=psO_bufs, space="PSUM"))

            def project_g(rows, dst_sb, col_lo, col_n, g):
                n_grp = (col_n + S_OWN - 1) // S_OWN
                for f in range(n_grp):
                    n0 = f * S_OWN
                    n1 = min(col_n, n0 + S_OWN)
                    ps = ppool.tile([P, S_OWN], F32, tag="qkps")
                    for c in range(8):
                        nc.tensor.matmul(
                            ps[:, : n1 - n0],
                            rows[c][:, g * P:(g + 1) * P],
                            xts[c][:, col_lo + n0: col_lo + n1],
                            start=(c == 0),
                            stop=(c == 7),
                        )
                    nc.vector.tensor_copy(dst_sb[:, g, n0:n1], ps[:, : n1 - n0])

            # score tiles packed in pairs sharing one PSUM bank: 4 exp ops
            # per head instead of 6 (less Act time + shorter e-ready chain)
            KT_PAIRS = [(0, 1), (2,), (3,), (4, 5)]

            def attend(h):
                g, r0 = h // 2, 64 * (h % 2)
                e_sb = epool.tile([P, 1536], ast, tag="e")
                for pair in KT_PAIRS:
                    psS = psS_pool.tile([P, 384], F32, tag="s")
                    loc = 0
                    for kt in pair:
                        lo, hi = KT_SPANS[kt]
                        N = KT_N[kt]
                        nc.tensor.matmul(
                            psS[:, loc:loc + N],
                            kT_sb[r0:r0 + 64, g, kt * P:(kt + 1) * P],
                            qT_sb[r0:r0 + 64, g, lo * P:(hi + 1) * P],
                            start=True,
                            stop=True,
                            skip_group_check=True,
                        )
                        loc += N
                    off = KT_OFF[pair[0]]
                    nc.scalar.activation(
                        e_sb[:, off:off + loc], psS[:, :loc],
                        mybir.ActivationFunctionType.Exp, scale=0.125,
                    )
                # triangles + sequence-edge validity in one multiplicative pass
                nc.vector.tensor_mul(e_sb[:], e_sb[:], trim_sb[:])

                psO = psO_pool.tile([65, S_OWN], F32, tag="o")
                for i, kt in enumerate([2, 0, 3, 5, 1, 4]):
                    lo, hi = KT_SPANS[kt]
                    off = KT_OFF[kt]
                    nc.tensor.matmul(
                        psO[:, lo * P:(hi + 1) * P],
                        v_sb[:, kt, 65 * h:65 * h + 65],
                        e_sb[:, off:off + KT_N[kt]],
                        start=(i == 0),
                        stop=(i == NKT - 1),
                        skip_group_check=True,
                    )
                recip = rpool.tile([1, S_OWN], F32, tag="r")
                nc.vector.reciprocal(recip[:], psO[64:65, :])
                bsb = rpool.tile([64, S_OWN], F32, tag="bs")
                nc.gpsimd.partition_broadcast(bsb[:], recip[:])
                nc.vector.tensor_mul(outT_g[g][r0:r0 + 64, :], psO[0:64, :], bsb[:])

            for g in range(8):
                project_g(wq_rows, qT_sb, HALO, S_OWN, g)
                project_g(wk_rows, kT_sb, 0, S_HALO, g)
                attend(2 * g)
                attend(2 * g + 1)

            # ================= output projection =================
            # psY reuses the qk projection ring (tag qkps): its first bank was
            # last drained by g7's k-proj, long before the final head's PV, so
            # the first O chain starts immediately after the last attention
            # matmul instead of waiting ~2.3us for a psO drain.
            # Two-pass output projection: pass 1 (groups 0-3) is runnable as
            # soon as heads 0-7 are normalized, so those chains flow through
            # the PSUM ring during the back half of attention and fill the
            # dead window while the final head's softmax normalize completes.
            # Pass 2 (groups 4-7) accumulates the rest; its drain is a DVE
            # add onto the pass-1 partial (no extra op on the tail).
            with tc.tile_pool(name="ysb", bufs=1) as ypool:
                half = S_OWN // 2
                y_acc = []
                for m in range(8):
                    psY = ppool.tile([P, S_OWN], F32, tag="qkps")
                    for c in range(4):
                        nc.tensor.matmul(
                            psY[:],
                            wo_rows[c][:, m * P:(m + 1) * P],
                            outT_g[c][:],
                            start=(c == 0),
                            stop=(c == 3),
                        )
                    ya = ypool.tile([P, S_OWN], F32, tag=f"yacc{m}", name=f"ya{m}")
                    if m % 2 == 0:
                        nc.scalar.copy(ya[:, :half], psY[:, :half])
                        nc.scalar.copy(ya[:, half:], psY[:, half:])
                    else:
                        nc.vector.tensor_copy(ya[:], psY[:])
                    y_acc.append(ya)
                for m in range(8):
                    psY = ppool.tile([P, S_OWN], F32, tag="qkps")
                    for c in range(4, 8):
                        nc.tensor.matmul(
                            psY[:],
                            wo_rows[c][:, m * P:(m + 1) * P],
                            outT_g[c][:],
                            start=(c == 4),
                            stop=(c == 7),
                        )
                    y_sb = ypool.tile([P, S_OWN], F32, tag="ysb", bufs=4)
                    nc.vector.tensor_add(y_sb[:], y_acc[m][:], psY[:])
                    ydst = yT.rearrange("(m p) n -> m p n", p=P)[m]
                    # one DMA per chunk, issue queues alternated: the 650ns
                    # per-dma_start sequencer cost was serializing the tail
                    eng = nc.sync if m % 2 == 0 else nc.scalar
                    eng.dma_start(ydst[:], y_sb[:])

        ctx2.close()

    nc.compile()
    return nc


def make_trimask(qs: int) -> np.ndarray:
    m = np.zeros((P, 1536), dtype=np.float32)
    for kt in range(NKT):
        lo, hi = KT_SPANS[kt]
        N = KT_N[kt]
        off = KT_OFF[kt]
        kj = np.arange(P)[:, None]
        f = np.arange(N)[None, :]
        qt = lo + f // P
        qi = f % P
        qg = qs + 128 * qt + qi
        kg = qs - 256 + 128 * kt + kj
        allowed = (kg >= 0) & (kg <= qg) & (kg >= qg - 255)
        m[:, off:off + N] = allowed.astype(np.float32)
    return m


def make_in_maps(x, wq, wk, wv, wo, proj_dt="bf16", att_dt="bf16"):
    npdt = _NP[proj_dt]
    andt = _NP[att_dt]
    wqT = np.ascontiguousarray(wq.T).astype(npdt)
    wkT = np.ascontiguousarray(wk.T).astype(npdt)
    wvT = np.ascontiguousarray(wv.T).astype(npdt)
    woT = np.ascontiguousarray(wo.T).astype(npdt)
    tri_edge = make_trimask(0).astype(andt)
    tri_mid = make_trimask(512).astype(andt)
    in_maps = []
    for c in range(8):
        b, qtr = c // 4, c % 4
        qs = 512 * qtr
        xh = np.zeros((S_HALO, D), dtype=np.float32)
        lo = qs - HALO
        xh[max(0, -lo):, :] = x[b, max(lo, 0): qs + S_OWN, :]
        in_maps.append({
            "xT": np.ascontiguousarray(xh.T).astype(npdt),
            "wqT": wqT, "wkT": wkT, "wvT": wvT, "woT": woT,
            "trimask7": tri_edge if qs == 0 else tri_mid,
        })
    return in_maps


def assemble_output(results):
    y = np.empty((2, 2048, D), dtype=np.float32)
    for c in range(8):
        b, qtr = c // 4, c % 4
        qs = 512 * qtr
        y[b, qs:qs + S_OWN, :] = results[c]["yT"].T
    return y


_NC_CACHE = {}
B, S = 2, 2048


def kernel(x, wq, wk, wv, wo):
    from concourse.bass_utils import run_bass_kernel_spmd
    x = np.asarray(x, dtype=np.float32)
    assert x.shape == (B, S, D), f"unexpected x shape {x.shape}"
    if "nc" not in _NC_CACHE:
        _NC_CACHE["nc"] = build_nc(psS_bufs=3, psO_bufs=2, qk_bufs=3)
    nc = _NC_CACHE["nc"]
    in_maps = make_in_maps(
        x, np.asarray(wq, np.float32), np.asarray(wk, np.float32),
        np.asarray(wv, np.float32), np.asarray(wo, np.float32))
    res = run_bass_kernel_spmd(nc, in_maps, core_ids=list(range(8)))
    return assemble_output(res.results)

